# revision 19
# baseline (speedup 1.0000x reference)
"""Trainium2 Bass kernel for nn_Attention_28862180229709.

Head-sharded (2 heads/core x 8 cores) fused attention:
  LayerNorm -> Q/KV projections -> interleaved RoPE -> per-head bilinear K
  transform -> softmax(QK^T)V -> output projection (row-parallel Wo),
  host-side sum of the 8 partial outputs.

Layout strategy (per core):
  - xn is transposed on-chip (PE transpose) to xnT [c, n] so all projections
    contract c on the partition axis.
  - q/k are produced transposed ([d, n]) with the head dims permuted to a
    global [h0-evens | h1-evens | h0-odds | h1-odds] row order so RoPE's
    interleaved pair-swap becomes a single 64-partition block swap
    (partner = row ^ 64) done with two strided DVE multiplies.
  - the per-head bilinear K transform is emitted as two scattered
    block-diagonal weight matmuls producing zero-padded ktT_h tensors, so
    the QK^T matmuls contract the full K=128 partition range (K=64 matmuls
    never warm the PE clock gate - measured 427ns vs 215ns at N=512).
  - scores are computed transposed (simT [keys, qrows]) so softmax
    normalization folds into the output side and attn @ V needs no
    transposes; row-sums come from an appended ones-column on V.
  - all matmuls run in bf16 (fp32 PSUM accumulation); LayerNorm, RoPE and
    softmax run in fp32 on DVE/ACT.
"""

import os
import sys

for _p in ("/opt/trn_rl_repo", "/root/.axon_site/_ro/trn_rl_repo"):
    if os.path.isdir(_p) and _p not in sys.path:
        sys.path.insert(0, _p)

from contextlib import ExitStack

import ml_dtypes
import numpy as np

import concourse.bacc as bacc
import concourse.tile as tile
from concourse import mybir
from concourse.bass_utils import run_bass_kernel_spmd

P = 128
DIM = 1024
HEADS = 16
DHEAD = 64
INNER = HEADS * DHEAD
NCORES = 8
HPC = HEADS // NCORES  # heads per core (2)
CB = DIM // P  # contraction chunks (8)
IB = 512  # i-block (psum bank) width
ROPE_BASE = 10000.0
LN_EPS = 1e-5

F32 = mybir.dt.float32
BF16 = mybir.dt.bfloat16
AF = mybir.ActivationFunctionType
ALU = mybir.AluOpType

# global q/k row order: [h0 evens | h1 evens | h0 odds | h1 odds].
# _QROWS[r] = (head, dim) of global row r; partner(r) = r ^ 64.
_EVENS = np.arange(0, DHEAD, 2)
_ODDS = np.arange(1, DHEAD, 2)


def _qcols():
    """Column indices (within a head-pair's 128 cols) for the global order."""
    cols = np.concatenate(
        [
            0 * DHEAD + _EVENS,
            1 * DHEAD + _EVENS,
            0 * DHEAD + _ODDS,
            1 * DHEAD + _ODDS,
        ]
    )
    return cols


def _head_rows(i):
    """Global q/k rows belonging to head i (i in 0,1)."""
    return np.concatenate([np.arange(32) + i * 32, np.arange(32) + 64 + i * 32])


def _build_nc(N, debug_taps=False):
    """Build the SPMD Bass program for sequence length N (tokens)."""
    NT = N // P  # token tiles
    NIB = N // IB  # i-blocks
    assert N % IB == 0

    nc = bacc.Bacc("TRN2", target_bir_lowering=False, debug=False)

    x_d = nc.dram_tensor("x", (N, DIM), F32, kind="ExternalInput")
    wq_d = nc.dram_tensor("wq", (CB, P, P), BF16, kind="ExternalInput")
    wk_d = nc.dram_tensor("wk", (CB, P, P), BF16, kind="ExternalInput")
    wv_d = nc.dram_tensor("wv", (CB, P, P), BF16, kind="ExternalInput")
    wb_d = nc.dram_tensor("wb", (HPC, P, P), BF16, kind="ExternalInput")
    wo_d = nc.dram_tensor("wo", (P, DIM), BF16, kind="ExternalInput")
    id_d = nc.dram_tensor("ident", (P, P), BF16, kind="ExternalInput")
    cos_d = nc.dram_tensor("cosT", (P, N), BF16, kind="ExternalInput")
    sin_d = nc.dram_tensor("sinT", (P, N), BF16, kind="ExternalInput")
    out_d = nc.dram_tensor("out", (N, DIM), F32, kind="ExternalOutput")
    warm_d = nc.dram_tensor("warm", (1, 1), F32, kind="ExternalOutput")
    if debug_taps:
        dbg = {
            "dbg_xnT": nc.dram_tensor("dbg_xnT", (P, CB, N), BF16, kind="ExternalOutput"),
            "dbg_q": nc.dram_tensor("dbg_q", (P, N), BF16, kind="ExternalOutput"),
            "dbg_k": nc.dram_tensor("dbg_k", (P, N), BF16, kind="ExternalOutput"),
            "dbg_kt": nc.dram_tensor("dbg_kt", (HPC, P, N), BF16, kind="ExternalOutput"),
            "dbg_e0": nc.dram_tensor("dbg_e0", (HPC, P, N), BF16, kind="ExternalOutput"),
            "dbg_r": nc.dram_tensor("dbg_r", (HPC, 1, N), F32, kind="ExternalOutput"),
            "dbg_osc": nc.dram_tensor("dbg_osc", (P, N), BF16, kind="ExternalOutput"),
            "dbg_v": nc.dram_tensor("dbg_v", (P, NT, 2 * (DHEAD + 1)), BF16, kind="ExternalOutput"),
        }

    VW = DHEAD + 1

    with tile.TileContext(nc) as tc, ExitStack() as ctx:
        const = ctx.enter_context(tc.tile_pool(name="const", bufs=1))
        big = ctx.enter_context(tc.tile_pool(name="big", bufs=1))

        wq_sb = const.tile([P, CB, P], BF16)
        wk_sb = const.tile([P, CB, P], BF16)
        wv_sb = const.tile([P, CB, P], BF16)
        wb_sb = const.tile([P, HPC, P], BF16)
        wo_sb = const.tile([P, DIM], BF16)
        id_sb = const.tile([P, P], BF16)
        cos_sb = const.tile([P, N], BF16)
        sin_sb = const.tile([P, N], BF16)
        eps_sb = const.tile([P, 1], F32)
        zero_sb = const.tile([P, 1], F32)
        nc.vector.memset(eps_sb[:], LN_EPS)
        nc.vector.memset(zero_sb[:], 0.0)
        warm_sb = const.tile([1, 1], F32)
        nc.scalar.activation(warm_sb[:], zero_sb[0:1, :], AF.Exp, bias=zero_sb[0:1, :])
        nc.sync.dma_start(warm_d[:], warm_sb[:])
        nc.sync.dma_start(wq_sb[:], wq_d[:].rearrange("a p m -> p a m"))
        nc.sync.dma_start(wk_sb[:], wk_d[:].rearrange("a p m -> p a m"))
        nc.sync.dma_start(wv_sb[:], wv_d[:].rearrange("a p m -> p a m"))
        nc.sync.dma_start(wb_sb[:], wb_d[:].rearrange("a p m -> p a m"))
        nc.sync.dma_start(wo_sb[:], wo_d[:])
        nc.sync.dma_start(id_sb[:], id_d[:])
        nc.sync.dma_start(cos_sb[:], cos_d[:])
        nc.sync.dma_start(sin_sb[:], sin_d[:])

        # long-lived activations
        xnT = big.tile([P, CB, N], BF16)  # xn transposed, c on partitions
        q_rope = big.tile([P, N], BF16)
        k_rope = big.tile([P, N], BF16)
        ktT = big.tile([P, HPC, N], BF16)  # zero-padded per head
        v_sb = big.tile([P, NT, HPC * VW], BF16)  # [keys, tile, head|ones]
        outT_sc = big.tile([P, N], BF16)  # scaled attn out, d on partitions

        nc.gpsimd.memset(v_sb[:], 1.0)

        # attention pools live in the outer scope so the scheduler can start
        # QK^T/exp work while the tail of the front is still running
        sps = ctx.enter_context(tc.tile_pool(name="sps", bufs=2, space="PSUM"))
        ep = ctx.enter_context(tc.tile_pool(name="ep", bufs=1))
        rp = ctx.enter_context(tc.tile_pool(name="rp", bufs=2))
        op = ctx.enter_context(tc.tile_pool(name="op", bufs=3))

        # ---- Front: LN + transpose + projections + rope + bilinear + v,
        # fused per token-group so everything pipelines ----
        with ExitStack() as actx:
            xp = actx.enter_context(tc.tile_pool(name="xp", bufs=3))
            sp = actx.enter_context(tc.tile_pool(name="sp", bufs=8))
            xnp = actx.enter_context(tc.tile_pool(name="xnp", bufs=4))
            rtmp = actx.enter_context(tc.tile_pool(name="rtmp", bufs=2))
            tp = actx.enter_context(tc.tile_pool(name="tp", bufs=1, space="PSUM"))
            qkps = actx.enter_context(tc.tile_pool(name="qkps", bufs=1, space="PSUM"))
            vps = actx.enter_context(tc.tile_pool(name="vps", bufs=1, space="PSUM"))

            n_group = IB // P  # token tiles per i-block group (4)
            for tg in range(NT // n_group):
                sl = slice(tg * IB, (tg + 1) * IB)
                # LayerNorm + PE transpose for this group's token tiles
                for ti in range(n_group):
                    t = tg * n_group + ti
                    xt = xp.tile([P, DIM], F32, tag="x")
                    nc.sync.dma_start(xt[:], x_d[t * P : (t + 1) * P, :])
                    st = sp.tile([P, 2, 6], F32, tag="st")
                    nc.vector.bn_stats(st[:, 0, :], xt[:, 0:512])
                    nc.vector.bn_stats(st[:, 1, :], xt[:, 512:1024])
                    mv = sp.tile([P, 2], F32, tag="mv")
                    nc.vector.bn_aggr(mv[:], st[:])
                    rstd = sp.tile([P, 1], F32, tag="rstd")
                    nc.scalar.activation(rstd[:], mv[:, 1:2], AF.Sqrt, bias=eps_sb[:])
                    nc.vector.reciprocal(rstd[:], rstd[:])
                    negmur = sp.tile([P, 1], F32, tag="negmur")
                    nc.vector.scalar_tensor_tensor(
                        negmur[:], mv[:, 0:1], -1.0, rstd[:], ALU.mult, ALU.mult
                    )
                    xn = xnp.tile([P, DIM], BF16, tag="xn")
                    nc.scalar.activation(
                        xn[:], xt[:], AF.Identity, bias=negmur[:], scale=rstd[:]
                    )
                    ps_t = [
                        tp.tile([P, 4, P], BF16, tag=f"t{half}", name=f"ps_t{half}")
                        for half in range(2)
                    ]
                    for cb in range(CB):
                        nc.tensor.transpose(
                            ps_t[cb // 4][:, cb % 4, :],
                            xn[:, cb * P : (cb + 1) * P],
                            id_sb[:],
                        )
                    for half in range(2):
                        nc.vector.tensor_copy(
                            xnT[:, 4 * half : 4 * half + 4, t * P : (t + 1) * P],
                            ps_t[half][:],
                        )
                # q/k projections + rope for this i-block
                for w_sb, dst in ((wq_sb, q_rope), (wk_sb, k_rope)):
                    ps_q = qkps.tile([P, IB], F32, tag="qk", name="ps_q")
                    for cb in range(CB):
                        nc.tensor.matmul(
                            ps_q[:],
                            w_sb[:, cb, :],
                            xnT[:, cb, sl],
                            start=(cb == 0),
                            stop=(cb == CB - 1),
                        )
                    tcos = rtmp.tile([P, IB], F32, tag="tcos")
                    nc.vector.tensor_mul(tcos[:], ps_q[:], cos_sb[:, sl])
                    tsin = rtmp.tile([P, IB], F32, tag="tsin")
                    nc.vector.tensor_mul(
                        tsin[0:64, :], ps_q[64:128, :], sin_sb[0:64, sl]
                    )
                    nc.vector.tensor_mul(
                        tsin[64:128, :], ps_q[0:64, :], sin_sb[64:128, sl]
                    )
                    nc.vector.tensor_add(dst[:, sl], tcos[:], tsin[:])
                # bilinear (zero-padded per head) for this i-block
                for h in range(HPC):
                    ps_kt = qkps.tile([P, IB], F32, tag="qk", name="ps_kt")
                    nc.tensor.matmul(
                        ps_kt[:], wb_sb[:, h, :], k_rope[:, sl], start=True, stop=True
                    )
                    nc.scalar.copy(ktT[:, h, sl], ps_kt[:])
                # v for this group's token tiles
                for ti in range(n_group):
                    t = tg * n_group + ti
                    ps_v = vps.tile([P, P], F32, tag="v")
                    for cb in range(CB):
                        nc.tensor.matmul(
                            ps_v[:],
                            xnT[:, cb, t * P : (t + 1) * P],
                            wv_sb[:, cb, :],
                            start=(cb == 0),
                            stop=(cb == CB - 1),
                        )
                    nc.scalar.copy(
                        v_sb[:, t, 0 : 2 * VW].rearrange("p (a b) -> p a b", a=2)[:, :, 0:DHEAD],
                        ps_v[:].rearrange("p (a b) -> p a b", a=2),
                    )
            if debug_taps:
                nc.sync.dma_start(dbg["dbg_k"][:], k_rope[:])

        # ---- Phase B: attention per head + interleaved output projection ----
        with ExitStack() as actx:
            avps = actx.enter_context(tc.tile_pool(name="avps", bufs=2, space="PSUM"))

            NG = 2 if NIB >= 2 else 1
            IPG = NIB // NG  # i-blocks per group
            GW = IPG * IB  # group width

            def wo_project(trange):
                """Output projection for token tiles in trange (needs outT_sc)."""
                for t in trange:
                    ps_o = avps.tile([P, DIM], F32, tag="av", name="ps_o")
                    for cc in range(DIM // IB):
                        nc.tensor.matmul(
                            ps_o[:, cc * IB : (cc + 1) * IB],
                            outT_sc[:, t * P : (t + 1) * P],
                            wo_sb[:, cc * IB : (cc + 1) * IB],
                            start=True,
                            stop=True,
                        )
                    o_sb = op.tile([P, DIM], F32, tag="osb")
                    nc.vector.tensor_copy(o_sb[:, 0:IB], ps_o[:, 0:IB])
                    nc.scalar.copy(o_sb[:, IB:DIM], ps_o[:, IB : 2 * IB])
                    nc.sync.dma_start(out_d[t * P : (t + 1) * P, :], o_sb[:])

            for h in range(HPC):
                expT = []
                for j in range(NT):
                    e_j = ep.tile([P, N], BF16, tag=f"e{j}", name="e_j")
                    for half in range(NG):
                        ps_s = sps.tile([P, GW], F32, tag="sim", name="ps_s")
                        for ibl in range(IPG):
                            gsl = slice(half * GW + ibl * IB, half * GW + (ibl + 1) * IB)
                            nc.tensor.matmul(
                                ps_s[:, ibl * IB : (ibl + 1) * IB],
                                ktT[:, h, j * P : (j + 1) * P],
                                q_rope[:, gsl],
                                start=True,
                                stop=True,
                            )
                        nc.scalar.activation(
                            e_j[:, half * GW : (half + 1) * GW],
                            ps_s[:],
                            AF.Exp,
                            bias=zero_sb[:],
                        )
                    if debug_taps and j == 0:
                        nc.sync.dma_start(dbg["dbg_e0"][h], e_j[:])
                    expT.append(e_j)
                def av_mms(grp, ps_av):
                    for j in range(NT):
                        for il in range(IPG):
                            esl = slice(grp * GW + il * IB, grp * GW + (il + 1) * IB)
                            nc.tensor.matmul(
                                ps_av[:, il * IB : (il + 1) * IB],
                                v_sb[:, j, h * VW : (h + 1) * VW],
                                expT[j][:, esl],
                                start=(j == 0),
                                stop=(j == NT - 1),
                            )

                def av_scale(grp, ps_av):
                    gsl = slice(grp * GW, (grp + 1) * GW)
                    rs_h = rp.tile([1, GW], F32, tag="rs")
                    nc.vector.tensor_copy(rs_h[:], ps_av[DHEAD : DHEAD + 1, :])
                    r_h = rp.tile([1, GW], F32, tag="r")
                    nc.vector.reciprocal_approx_fast(r_h[:], rs_h[:])
                    if debug_taps:
                        nc.sync.dma_start(dbg["dbg_r"][h, :, gsl], r_h[:])
                    rb_h = rp.tile([P, GW], F32, tag="rb")
                    nc.gpsimd.partition_broadcast(rb_h[:], r_h[:])
                    nc.vector.tensor_mul(
                        outT_sc[h * DHEAD : (h + 1) * DHEAD, gsl],
                        ps_av[0:DHEAD, :],
                        rb_h[h * DHEAD : (h + 1) * DHEAD, :],
                    )

                ps_avs = [
                    avps.tile([DHEAD + 1, GW], F32, tag="av", name=f"ps_av{g}")
                    for g in range(NG)
                ]
                for grp in range(NG):
                    av_mms(grp, ps_avs[grp])
                    av_scale(grp, ps_avs[grp])
                    if h == HPC - 1:
                        if debug_taps and grp == NG - 1:
                            nc.sync.dma_start(dbg["dbg_osc"][:], outT_sc[:])
                        tpg = NT // NG
                        wo_project(range(grp * tpg, (grp + 1) * tpg))

    nc.compile()
    return nc


def _rope_tables(N):
    theta = 1.0 / (ROPE_BASE ** (np.arange(0, DHEAD, 2, dtype=np.float64) / DHEAD))
    pos = np.arange(N, dtype=np.float64)
    freqs = pos[:, None] * theta[None, :]  # [N, 32]
    emb = np.concatenate([freqs, freqs], axis=-1)  # [N, 64]
    cos, sin = np.cos(emb), np.sin(emb)  # [N, 64]
    # tables in the global row order [h0e | h1e | h0o | h1o]:
    # row r (even block): coefficient of dim 2r'; odd block: dim 2r'+1.
    cosT = np.empty((P, N))
    sinT = np.empty((P, N))
    for hb in range(2):  # which head's 32-block within each half
        for r in range(32):
            cosT[hb * 32 + r] = cos[:, 2 * r]
            cosT[64 + hb * 32 + r] = cos[:, 2 * r + 1]
            # out_even = q_even*cos - q_odd*sin ; out_odd = q_odd*cos + q_even*sin
            sinT[hb * 32 + r] = -sin[:, 2 * r]
            sinT[64 + hb * 32 + r] = sin[:, 2 * r + 1]
    return (
        np.ascontiguousarray(cosT.astype(ml_dtypes.bfloat16)),
        np.ascontiguousarray(sinT.astype(ml_dtypes.bfloat16)),
    )


def _prep_inputs(x, gamma, Wq, Wkv, W_bilinear, Wo):
    """Slice/permute weights per core; returns list of 8 input dicts."""
    b, N, _ = x.shape
    x2d = np.ascontiguousarray(x.reshape(N, DIM)).astype(np.float32)
    cosT, sinT = _rope_tables(N)
    ident = np.eye(P, dtype=ml_dtypes.bfloat16)

    g = gamma.astype(np.float64)
    Wqg = g[:, None] * Wq.astype(np.float64) * (DHEAD**-0.5)
    Wkg = g[:, None] * Wkv[:, :INNER].astype(np.float64)
    Wvg = g[:, None] * Wkv[:, INNER:].astype(np.float64)

    qcols = _qcols()
    in_maps = []
    for c in range(NCORES):
        heads = [HPC * c + i for i in range(HPC)]
        # columns of the head-pair in global row order
        pair_cols = np.concatenate([np.arange(h * DHEAD, (h + 1) * DHEAD) for h in heads])
        gq = pair_cols[qcols]  # global row r <- original inner column gq[r]
        vcols = pair_cols
        wq_c = Wqg[:, gq].astype(ml_dtypes.bfloat16).reshape(CB, P, P)
        wk_c = Wkg[:, gq].astype(ml_dtypes.bfloat16).reshape(CB, P, P)
        wv_c = Wvg[:, vcols].astype(ml_dtypes.bfloat16).reshape(CB, P, P)
        # scattered block-diagonal bilinear weights, zero-padded per head:
        # row r (k_rope row, dim dk), col e (ktT row, dim de) nonzero only for
        # rows/cols of head h: wb[h][r, e] = W_bilinear[head][dk, de]
        wb_c = np.zeros((HPC, P, P), dtype=np.float64)
        dim_of_row = np.empty(P, dtype=np.int64)
        head_of_row = np.empty(P, dtype=np.int64)
        for i in range(HPC):
            rows = _head_rows(i)
            dims = np.concatenate([_EVENS, _ODDS])
            dim_of_row[rows] = dims
            head_of_row[rows] = i
        for i, h in enumerate(heads):
            rows = _head_rows(i)
            wb_h = W_bilinear[h].astype(np.float64)
            sub = wb_h[np.ix_(dim_of_row[rows], dim_of_row[rows])]
            wb_c[i][np.ix_(rows, rows)] = sub
        wo_c = Wo[vcols, :].astype(ml_dtypes.bfloat16)
        in_maps.append(
            {
                "x": x2d,
                "wq": np.ascontiguousarray(wq_c),
                "wk": np.ascontiguousarray(wk_c),
                "wv": np.ascontiguousarray(wv_c),
                "wb": np.ascontiguousarray(wb_c.astype(ml_dtypes.bfloat16)),
                "wo": np.ascontiguousarray(wo_c),
                "cosT": cosT,
                "sinT": sinT,
                "ident": ident,
            }
        )
    return in_maps


_NC_CACHE = {}


def _get_nc(N):
    if N not in _NC_CACHE:
        _NC_CACHE[N] = _build_nc(N)
    return _NC_CACHE[N]


def kernel(x, gamma, Wq, Wkv, W_bilinear, Wo, _trace=False, _trace_kwargs=None):
    b, N, dim = x.shape
    assert b == 1 and dim == DIM
    nc = _get_nc(N)
    in_maps = _prep_inputs(x, gamma, Wq, Wkv, W_bilinear, Wo)
    kw = {}
    if _trace:
        kw = {"trace": True, **(_trace_kwargs or {})}
    res = run_bass_kernel_spmd(nc, in_maps, core_ids=list(range(NCORES)), **kw)
    acc = np.zeros((N, DIM), dtype=np.float64)
    for c in range(NCORES):
        acc += res.results[c]["out"].astype(np.float64)
    out = acc.astype(np.float32).reshape(1, N, DIM)
    if _trace:
        return out, res
    return out


# revision 25
# speedup vs baseline: 1.0130x; 1.0130x over previous
"""Trainium2 Bass kernel for nn_Attention_28862180229709.

Head-sharded (2 heads/core x 8 cores) fused attention:
  LayerNorm -> Q/KV projections -> interleaved RoPE -> per-head bilinear K
  transform -> softmax(QK^T)V -> output projection (row-parallel Wo),
  host-side sum of the 8 partial outputs.

Layout strategy (per core):
  - xn is transposed on-chip (PE transpose) to xnT [c, n] so all projections
    contract c on the partition axis.
  - q/k are produced transposed ([d, n]) with each head's dims permuted to
    [evens | odds] so RoPE's interleaved pair-swap becomes 32-partition
    block swaps (partner = row ^ 32) done with strided DVE multiplies.
  - the per-head bilinear K transform is a single scattered block-diagonal
    weight matmul; QK^T runs as per-head K=64 matmul PAIRS packed into the
    PE array with tile_position row groups (a lone K=64 matmul never warms
    the PE clock gate - measured 427ns vs 216ns at N=512; a packed pair
    runs both heads in ~342ns).
  - scores are computed transposed (simT [keys, qrows]) so softmax
    normalization folds into the output side and attn @ V needs no
    transposes; row-sums come from an appended ones-column on V.
  - all matmuls run in bf16 (fp32 PSUM accumulation); LayerNorm, RoPE and
    softmax run in fp32 on DVE/ACT.
"""

import os
import sys

for _p in ("/opt/trn_rl_repo", "/root/.axon_site/_ro/trn_rl_repo"):
    if os.path.isdir(_p) and _p not in sys.path:
        sys.path.insert(0, _p)

from contextlib import ExitStack

import ml_dtypes
import numpy as np

import concourse.bacc as bacc
import concourse.tile as tile
from concourse import mybir
from concourse.bass_utils import run_bass_kernel_spmd

P = 128
DIM = 1024
HEADS = 16
DHEAD = 64
INNER = HEADS * DHEAD
NCORES = 8
HPC = HEADS // NCORES  # heads per core (2)
CB = DIM // P  # contraction chunks (8)
IB = 512  # i-block (psum bank) width
ROPE_BASE = 10000.0
LN_EPS = 1e-5

F32 = mybir.dt.float32
BF16 = mybir.dt.bfloat16
AF = mybir.ActivationFunctionType
ALU = mybir.AluOpType

# q/k row order: per head [evens(32) | odds(32)], heads contiguous.
_EVENS = np.arange(0, DHEAD, 2)
_ODDS = np.arange(1, DHEAD, 2)


def _build_nc(N, debug_taps=False):
    """Build the SPMD Bass program for sequence length N (tokens)."""
    NT = N // P  # token tiles
    NIB = N // IB  # i-blocks
    assert N % IB == 0

    nc = bacc.Bacc("TRN2", target_bir_lowering=False, debug=False, dynamic_dma_scratch_size=2048)

    x_d = nc.dram_tensor("x", (N, DIM), F32, kind="ExternalInput")
    wq_d = nc.dram_tensor("wq", (CB, P, P), BF16, kind="ExternalInput")
    wk_d = nc.dram_tensor("wk", (CB, P, P), BF16, kind="ExternalInput")
    wv_d = nc.dram_tensor("wv", (CB, P, P), BF16, kind="ExternalInput")
    wb_d = nc.dram_tensor("wb", (P, P), BF16, kind="ExternalInput")
    wo_d = nc.dram_tensor("wo", (P, DIM), BF16, kind="ExternalInput")
    id_d = nc.dram_tensor("ident", (P, P), BF16, kind="ExternalInput")
    cos_d = nc.dram_tensor("cosT", (P, N), BF16, kind="ExternalInput")
    sin_d = nc.dram_tensor("sinT", (P, N), BF16, kind="ExternalInput")
    out_d = nc.dram_tensor("out", (N, DIM), F32, kind="ExternalOutput")
    warm_d = nc.dram_tensor("warm", (1, 1), F32, kind="ExternalOutput")
    if debug_taps:
        dbg = {
            "dbg_xnT": nc.dram_tensor("dbg_xnT", (P, CB, N), BF16, kind="ExternalOutput"),
            "dbg_q": nc.dram_tensor("dbg_q", (P, N), BF16, kind="ExternalOutput"),
            "dbg_k": nc.dram_tensor("dbg_k", (P, N), BF16, kind="ExternalOutput"),
            "dbg_kt": nc.dram_tensor("dbg_kt", (P, N), BF16, kind="ExternalOutput"),
            "dbg_r": nc.dram_tensor("dbg_r", (HPC, 1, N), F32, kind="ExternalOutput"),
            "dbg_osc": nc.dram_tensor("dbg_osc", (P, N), BF16, kind="ExternalOutput"),
            "dbg_v": nc.dram_tensor("dbg_v", (P, NT, 2 * (DHEAD + 1)), BF16, kind="ExternalOutput"),
        }

    VW = DHEAD + 1

    with tile.TileContext(nc) as tc, ExitStack() as ctx:
        const = ctx.enter_context(tc.tile_pool(name="const", bufs=1))
        big = ctx.enter_context(tc.tile_pool(name="big", bufs=1))

        wq_sb = const.tile([P, CB, P], BF16)
        wk_sb = const.tile([P, CB, P], BF16)
        wv_sb = const.tile([P, CB, P], BF16)
        wb_sb = const.tile([P, P], BF16)
        wo_sb = const.tile([P, DIM], BF16)
        id_sb = const.tile([P, P], BF16)
        cos_sb = const.tile([P, N], BF16)
        sin_sb = const.tile([P, N], BF16)
        eps_sb = const.tile([P, 1], F32)
        zero_sb = const.tile([P, 1], F32)
        nc.vector.memset(eps_sb[:], LN_EPS)
        nc.vector.memset(zero_sb[:], 0.0)
        # touch Exp early so the ACT table load lands in the DMA bubble
        warm_sb = const.tile([1, 1], F32)
        nc.scalar.activation(warm_sb[:], zero_sb[0:1, :], AF.Exp, bias=zero_sb[0:1, :])
        nc.sync.dma_start(warm_d[:], warm_sb[:])
        nc.sync.dma_start(wq_sb[:], wq_d[:].rearrange("a p m -> p a m"))
        nc.sync.dma_start(wk_sb[:], wk_d[:].rearrange("a p m -> p a m"))
        nc.sync.dma_start(wv_sb[:], wv_d[:].rearrange("a p m -> p a m"))
        nc.sync.dma_start(wb_sb[:], wb_d[:])
        nc.sync.dma_start(wo_sb[:], wo_d[:])
        nc.sync.dma_start(id_sb[:], id_d[:])
        nc.sync.dma_start(cos_sb[:], cos_d[:])
        nc.sync.dma_start(sin_sb[:], sin_d[:])

        # long-lived activations
        xnT = big.tile([P, CB, N], BF16)  # xn transposed, c on partitions
        q_rope = big.tile([P, N], BF16)
        k_rope = big.tile([P, N], BF16)
        ktT = big.tile([P, N], BF16)  # heads contiguous on partitions
        v_sb = big.tile([P, NT, HPC * VW], BF16)  # [keys, tile, head|ones]
        outT_sc = big.tile([P, N], BF16)  # scaled attn out, d on partitions

        nc.gpsimd.memset(v_sb[:], 1.0)

        # ---- Front: LN + transpose + projections + rope + bilinear + v ----
        with ExitStack() as actx:
            xp = actx.enter_context(tc.tile_pool(name="xp", bufs=5))
            sp = actx.enter_context(tc.tile_pool(name="sp", bufs=8))
            xnp = actx.enter_context(tc.tile_pool(name="xnp", bufs=4))
            rtmp = actx.enter_context(tc.tile_pool(name="rtmp", bufs=4))
            tp = actx.enter_context(tc.tile_pool(name="tp", bufs=1, space="PSUM"))
            qkps = actx.enter_context(tc.tile_pool(name="qkps", bufs=2, space="PSUM"))
            vps = actx.enter_context(tc.tile_pool(name="vps", bufs=2, space="PSUM"))

            n_group = IB // P  # token tiles per i-block group (4)
            for tg in range(NT // n_group):
                sl = slice(tg * IB, (tg + 1) * IB)
                # LayerNorm + PE transpose for this group's token tiles
                ps_t = [
                    tp.tile([P, 2, IB], BF16, tag=f"t{cbp}", name=f"ps_t{cbp}")
                    for cbp in range(CB // 2)
                ]
                for ti in range(n_group):
                    t = tg * n_group + ti
                    xt = xp.tile([P, DIM], F32, tag="x")
                    nc.sync.dma_start(xt[:], x_d[t * P : (t + 1) * P, :])
                    st = sp.tile([P, 2, 6], F32, tag="st")
                    nc.vector.bn_stats(st[:, 0, :], xt[:, 0:512])
                    nc.vector.bn_stats(st[:, 1, :], xt[:, 512:1024])
                    mv = sp.tile([P, 2], F32, tag="mv")
                    nc.vector.bn_aggr(mv[:], st[:])
                    rstd = sp.tile([P, 1], F32, tag="rstd")
                    nc.scalar.activation(rstd[:], mv[:, 1:2], AF.Sqrt, bias=eps_sb[:])
                    nc.vector.reciprocal(rstd[:], rstd[:])
                    xn = xnp.tile([P, DIM], BF16, tag="xn")
                    nc.vector.tensor_scalar(
                        xn[:], xt[:], mv[:, 0:1], rstd[:], ALU.subtract, ALU.mult
                    )
                    for cb in range(CB):
                        nc.tensor.transpose(
                            ps_t[cb // 2][:, cb % 2, ti * P : (ti + 1) * P],
                            xn[:, cb * P : (cb + 1) * P],
                            id_sb[:],
                        )
                for cbp in range(CB // 2):
                    nc.scalar.copy(xnT[:, 2 * cbp : 2 * cbp + 2, sl], ps_t[cbp][:])
                # q/k projections + rope for this i-block
                for w_sb, dst in ((wq_sb, q_rope), (wk_sb, k_rope)):
                    ps_q = qkps.tile([P, IB], F32, tag="qk", name="ps_q")
                    for cb in range(CB):
                        nc.tensor.matmul(
                            ps_q[:],
                            w_sb[:, cb, :],
                            xnT[:, cb, sl],
                            start=(cb == 0),
                            stop=(cb == CB - 1),
                        )
                    tcos = rtmp.tile([P, IB], F32, tag="tcos")
                    nc.vector.tensor_mul(tcos[:], ps_q[:], cos_sb[:, sl])
                    tsin = rtmp.tile([P, IB], F32, tag="tsin")
                    for blk in range(4):
                        o0 = blk * 32
                        i0 = (blk ^ 1) * 32
                        nc.vector.tensor_mul(
                            tsin[o0 : o0 + 32, :],
                            ps_q[i0 : i0 + 32, :],
                            sin_sb[o0 : o0 + 32, sl],
                        )
                    nc.vector.tensor_add(dst[:, sl], tcos[:], tsin[:])
                # bilinear (block-diagonal, both heads) for this i-block
                ps_kt = qkps.tile([P, IB], F32, tag="qk", name="ps_kt")
                nc.tensor.matmul(
                    ps_kt[:], wb_sb[:], k_rope[:, sl], start=True, stop=True
                )
                nc.scalar.copy(ktT[:, sl], ps_kt[:])
                # v for this group's token tiles
                for ti in range(n_group):
                    t = tg * n_group + ti
                    ps_v = vps.tile([P, P], F32, tag="v")
                    for cb in range(CB):
                        nc.tensor.matmul(
                            ps_v[:],
                            xnT[:, cb, t * P : (t + 1) * P],
                            wv_sb[:, cb, :],
                            start=(cb == 0),
                            stop=(cb == CB - 1),
                        )
                    nc.scalar.copy(
                        v_sb[:, t, 0 : 2 * VW].rearrange("p (a b) -> p a b", a=2)[
                            :, :, 0:DHEAD
                        ],
                        ps_v[:].rearrange("p (a b) -> p a b", a=2),
                    )
            if debug_taps:
                nc.sync.dma_start(dbg["dbg_k"][:], k_rope[:])

        if debug_taps:
            nc.sync.dma_start(dbg["dbg_xnT"][:], xnT[:])
            nc.sync.dma_start(dbg["dbg_q"][:], q_rope[:])
            nc.sync.dma_start(dbg["dbg_kt"][:], ktT[:])
            nc.sync.dma_start(dbg["dbg_v"][:], v_sb[:])

        # ---- Attention: paired-head QK^T + exp stream, AV per head,
        # output projection interleaved per i-block group ----
        with ExitStack() as actx:
            sps = actx.enter_context(tc.tile_pool(name="sps", bufs=2, space="PSUM"))
            avps = actx.enter_context(tc.tile_pool(name="avps", bufs=2, space="PSUM"))
            ep = actx.enter_context(tc.tile_pool(name="ep", bufs=1))
            rp = actx.enter_context(tc.tile_pool(name="rp", bufs=1))
            op = actx.enter_context(tc.tile_pool(name="op", bufs=2))

            NG = 2 if NIB >= 2 else 1
            IPG = NIB // NG  # i-blocks per group
            GW = IPG * IB  # group width

            def wo_project(trange):
                """Output projection for token tiles in trange (needs outT_sc)."""
                for t in trange:
                    ps_o = sps.tile([P, HPC, IB], F32, tag="sim", name="ps_o")
                    for cc in range(DIM // IB):
                        nc.tensor.matmul(
                            ps_o[:, cc, :],
                            outT_sc[:, t * P : (t + 1) * P],
                            wo_sb[:, cc * IB : (cc + 1) * IB],
                            start=True,
                            stop=True,
                        )
                    o_lo = op.tile([P, IB], F32, tag="osb")
                    nc.vector.tensor_copy(o_lo[:], ps_o[:, 0, :])
                    nc.sync.dma_start(out_d[t * P : (t + 1) * P, 0:IB], o_lo[:])
                    o_hi = op.tile([P, IB], F32, tag="osb2")
                    nc.scalar.copy(o_hi[:], ps_o[:, 1, :])
                    nc.sync.dma_start(out_d[t * P : (t + 1) * P, IB:DIM], o_hi[:])

            # paired-head sim + exp stream over all (j, ib)
            expT = []
            for j in range(NT):
                e_j = ep.tile([P, NIB, HPC, IB], BF16, tag=f"e{j}", name="e_j")
                for ib in range(NIB):
                    sl = slice(ib * IB, (ib + 1) * IB)
                    ps_s = sps.tile([P, HPC, IB], F32, tag="sim", name="ps_s")
                    for h in range(HPC):
                        hl = slice(h * DHEAD, (h + 1) * DHEAD)
                        nc.tensor.matmul(
                            ps_s[:, h, :],
                            ktT[hl, j * P : (j + 1) * P],
                            q_rope[hl, sl],
                            start=True,
                            stop=True,
                        )
                    nc.scalar.activation(
                        e_j[:, ib, :, :], ps_s[:], AF.Exp, bias=zero_sb[:]
                    )
                expT.append(e_j)

            for h in range(HPC):

                def av_mms(grp, ps_av):
                    for j in range(NT):
                        for il in range(IPG):
                            ib = grp * IPG + il
                            nc.tensor.matmul(
                                ps_av[:, il * IB : (il + 1) * IB],
                                v_sb[:, j, h * VW : (h + 1) * VW],
                                expT[j][:, ib, h, :],
                                start=(j == 0),
                                stop=(j == NT - 1),
                            )

                def av_scale(grp, ps_av):
                    for il in range(IPG):
                        gsl = slice(grp * GW + il * IB, grp * GW + (il + 1) * IB)
                        lsl = slice(il * IB, (il + 1) * IB)
                        rs_h = rp.tile([1, IB], F32, tag="rs")
                        nc.vector.tensor_copy(rs_h[:], ps_av[DHEAD : DHEAD + 1, lsl])
                        r_h = rp.tile([1, IB], F32, tag="r")
                        nc.vector.reciprocal_approx_fast(r_h[:], rs_h[:])
                        if debug_taps:
                            nc.sync.dma_start(dbg["dbg_r"][h, :, gsl], r_h[:])
                        rb_h = rp.tile([P, IB], F32, tag="rb")
                        nc.gpsimd.partition_broadcast(rb_h[:], r_h[:])
                        nc.vector.tensor_mul(
                            outT_sc[h * DHEAD : (h + 1) * DHEAD, gsl],
                            ps_av[0:DHEAD, lsl],
                            rb_h[h * DHEAD : (h + 1) * DHEAD, :],
                        )

                ps_avs = [
                    avps.tile([DHEAD + 1, GW], F32, tag="av", name=f"ps_av{g}")
                    for g in range(NG)
                ]
                for grp in range(NG):
                    av_mms(grp, ps_avs[grp])
                    av_scale(grp, ps_avs[grp])
                    if h == HPC - 1:
                        if debug_taps and grp == NG - 1:
                            nc.sync.dma_start(dbg["dbg_osc"][:], outT_sc[:])
                        tpg = NT // NG
                        wo_project(range(grp * tpg, (grp + 1) * tpg))

    nc.compile()
    return nc


def _rope_tables(N):
    theta = 1.0 / (ROPE_BASE ** (np.arange(0, DHEAD, 2, dtype=np.float64) / DHEAD))
    pos = np.arange(N, dtype=np.float64)
    freqs = pos[:, None] * theta[None, :]  # [N, 32]
    emb = np.concatenate([freqs, freqs], axis=-1)  # [N, 64]
    cos, sin = np.cos(emb), np.sin(emb)  # [N, 64]
    # per-head row order [evens(32) | odds(32)]:
    # out_even = q_even*cos[2r] - q_odd*sin[2r]
    # out_odd  = q_odd*cos[2r+1] + q_even*sin[2r+1]
    cosT = np.empty((DHEAD, N))
    sinT = np.empty((DHEAD, N))
    for r in range(32):
        cosT[r] = cos[:, 2 * r]
        cosT[32 + r] = cos[:, 2 * r + 1]
        sinT[r] = -sin[:, 2 * r]
        sinT[32 + r] = sin[:, 2 * r + 1]
    cosT2 = np.concatenate([cosT, cosT], axis=0)
    sinT2 = np.concatenate([sinT, sinT], axis=0)
    return (
        np.ascontiguousarray(cosT2.astype(ml_dtypes.bfloat16)),
        np.ascontiguousarray(sinT2.astype(ml_dtypes.bfloat16)),
    )


def _prep_inputs(x, gamma, Wq, Wkv, W_bilinear, Wo):
    """Slice/permute weights per core; returns list of 8 input dicts."""
    b, N, _ = x.shape
    x2d = np.ascontiguousarray(x.reshape(N, DIM)).astype(np.float32)
    cosT, sinT = _rope_tables(N)
    ident = np.eye(P, dtype=ml_dtypes.bfloat16)

    g = gamma.astype(np.float64)
    Wqg = g[:, None] * Wq.astype(np.float64) * (DHEAD**-0.5)
    Wkg = g[:, None] * Wkv[:, :INNER].astype(np.float64)
    Wvg = g[:, None] * Wkv[:, INNER:].astype(np.float64)

    perm = np.concatenate([_EVENS, _ODDS])  # within-head row order
    in_maps = []
    for c in range(NCORES):
        heads = [HPC * c + i for i in range(HPC)]
        gq = np.concatenate([h * DHEAD + perm for h in heads])
        vcols = np.concatenate(
            [np.arange(h * DHEAD, (h + 1) * DHEAD) for h in heads]
        )
        wq_c = Wqg[:, gq].astype(ml_dtypes.bfloat16).reshape(CB, P, P)
        wk_c = Wkg[:, gq].astype(ml_dtypes.bfloat16).reshape(CB, P, P)
        wv_c = Wvg[:, vcols].astype(ml_dtypes.bfloat16).reshape(CB, P, P)
        # block-diagonal bilinear: rows = k_rope rows, cols = ktT rows,
        # both in per-head [evens|odds] order
        wb_c = np.zeros((P, P), dtype=np.float64)
        for i, h in enumerate(heads):
            rows = np.arange(i * DHEAD, (i + 1) * DHEAD)
            wb_h = W_bilinear[h].astype(np.float64)[np.ix_(perm, perm)]
            wb_c[np.ix_(rows, rows)] = wb_h
        wo_c = Wo[vcols, :].astype(ml_dtypes.bfloat16)
        in_maps.append(
            {
                "x": x2d,
                "wq": np.ascontiguousarray(wq_c),
                "wk": np.ascontiguousarray(wk_c),
                "wv": np.ascontiguousarray(wv_c),
                "wb": np.ascontiguousarray(wb_c.astype(ml_dtypes.bfloat16)),
                "wo": np.ascontiguousarray(wo_c),
                "ident": ident,
                "cosT": cosT,
                "sinT": sinT,
            }
        )
    return in_maps


_NC_CACHE = {}


def _get_nc(N):
    if N not in _NC_CACHE:
        _NC_CACHE[N] = _build_nc(N)
    return _NC_CACHE[N]


def kernel(x, gamma, Wq, Wkv, W_bilinear, Wo, _trace=False, _trace_kwargs=None):
    b, N, dim = x.shape
    assert b == 1 and dim == DIM
    nc = _get_nc(N)
    in_maps = _prep_inputs(x, gamma, Wq, Wkv, W_bilinear, Wo)
    kw = {}
    if _trace:
        kw = {"trace": True, **(_trace_kwargs or {})}
    res = run_bass_kernel_spmd(nc, in_maps, core_ids=list(range(NCORES)), **kw)
    acc = np.zeros((N, DIM), dtype=np.float64)
    for c in range(NCORES):
        acc += res.results[c]["out"].astype(np.float64)
    out = acc.astype(np.float32).reshape(1, N, DIM)
    if _trace:
        return out, res
    return out


# revision 29
# speedup vs baseline: 1.0240x; 1.0109x over previous
"""Trainium2 Bass kernel for nn_Attention_28862180229709.

Head-sharded (2 heads/core x 8 cores) fused attention:
  LayerNorm -> Q/KV projections -> interleaved RoPE -> per-head bilinear K
  transform -> softmax(QK^T)V -> output projection (row-parallel Wo),
  host-side sum of the 8 partial outputs.

Layout strategy (per core):
  - xn is transposed on-chip (PE transpose) to xnT [c, n] so all projections
    contract c on the partition axis.
  - q/k are produced transposed ([d, n]) with each head's dims permuted to
    [evens | odds] so RoPE's interleaved pair-swap becomes 32-partition
    block swaps (partner = row ^ 32) done with strided DVE multiplies.
  - the per-head bilinear K transform is a single scattered block-diagonal
    weight matmul; QK^T runs as per-head K=64 matmul PAIRS packed into the
    PE array with tile_position row groups (a lone K=64 matmul never warms
    the PE clock gate - measured 427ns vs 216ns at N=512; a packed pair
    runs both heads in ~342ns).
  - scores are computed transposed (simT [keys, qrows]) so softmax
    normalization folds into the output side and attn @ V needs no
    transposes; row-sums come from an appended ones-column on V.
  - all matmuls run in bf16 (fp32 PSUM accumulation); LayerNorm, RoPE and
    softmax run in fp32 on DVE/ACT.
"""

import os
import sys

for _p in ("/opt/trn_rl_repo", "/root/.axon_site/_ro/trn_rl_repo"):
    if os.path.isdir(_p) and _p not in sys.path:
        sys.path.insert(0, _p)

from contextlib import ExitStack

import ml_dtypes
import numpy as np

import concourse.bacc as bacc
import concourse.tile as tile
from concourse import mybir
from concourse.bass_utils import run_bass_kernel_spmd

P = 128
DIM = 1024
HEADS = 16
DHEAD = 64
INNER = HEADS * DHEAD
NCORES = 8
HPC = HEADS // NCORES  # heads per core (2)
CB = DIM // P  # contraction chunks (8)
IB = 512  # i-block (psum bank) width
ROPE_BASE = 10000.0
LN_EPS = 1e-5

F32 = mybir.dt.float32
BF16 = mybir.dt.bfloat16
AF = mybir.ActivationFunctionType
ALU = mybir.AluOpType

# q/k row order: per head [evens(32) | odds(32)], heads contiguous.
_EVENS = np.arange(0, DHEAD, 2)
_ODDS = np.arange(1, DHEAD, 2)


def _build_nc(N, debug_taps=False):
    """Build the SPMD Bass program for sequence length N (tokens)."""
    NT = N // P  # token tiles
    NIB = N // IB  # i-blocks
    assert N % IB == 0

    nc = bacc.Bacc("TRN2", target_bir_lowering=False, debug=False, dynamic_dma_scratch_size=2048)

    x_d = nc.dram_tensor("x", (N, DIM), F32, kind="ExternalInput")
    wq_d = nc.dram_tensor("wq", (CB, P, P), BF16, kind="ExternalInput")
    wk_d = nc.dram_tensor("wk", (CB, P, P), BF16, kind="ExternalInput")
    wv_d = nc.dram_tensor("wv", (CB, P, P), BF16, kind="ExternalInput")
    wb_d = nc.dram_tensor("wb", (P, P), BF16, kind="ExternalInput")
    wo_d = nc.dram_tensor("wo", (P, DIM), BF16, kind="ExternalInput")
    id_d = nc.dram_tensor("ident", (P, P), BF16, kind="ExternalInput")
    cos_d = nc.dram_tensor("cosT", (P, N), BF16, kind="ExternalInput")
    sin_d = nc.dram_tensor("sinT", (P, N), BF16, kind="ExternalInput")
    out_d = nc.dram_tensor("out", (N, DIM), F32, kind="ExternalOutput")
    warm_d = nc.dram_tensor("warm", (1, 1), F32, kind="ExternalOutput")
    if debug_taps:
        dbg = {
            "dbg_xnT": nc.dram_tensor("dbg_xnT", (P, CB, N), BF16, kind="ExternalOutput"),
            "dbg_q": nc.dram_tensor("dbg_q", (P, N), BF16, kind="ExternalOutput"),
            "dbg_k": nc.dram_tensor("dbg_k", (P, N), BF16, kind="ExternalOutput"),
            "dbg_kt": nc.dram_tensor("dbg_kt", (P, N), BF16, kind="ExternalOutput"),
            "dbg_r": nc.dram_tensor("dbg_r", (HPC, 1, N), F32, kind="ExternalOutput"),
            "dbg_osc": nc.dram_tensor("dbg_osc", (P, N), BF16, kind="ExternalOutput"),
            "dbg_v": nc.dram_tensor("dbg_v", (P, NT, 2 * (DHEAD + 1)), BF16, kind="ExternalOutput"),
        }

    VW = DHEAD + 1

    with tile.TileContext(nc) as tc, ExitStack() as ctx:
        const = ctx.enter_context(tc.tile_pool(name="const", bufs=1))
        big = ctx.enter_context(tc.tile_pool(name="big", bufs=1))

        wq_sb = const.tile([P, CB, P], BF16)
        wk_sb = const.tile([P, CB, P], BF16)
        wv_sb = const.tile([P, CB, P], BF16)
        wb_sb = const.tile([P, P], BF16)
        wo_sb = const.tile([P, DIM], BF16)
        id_sb = const.tile([P, P], BF16)
        cos_sb = const.tile([P, N], BF16)
        sin_sb = const.tile([P, N], BF16)
        eps_sb = const.tile([P, 1], F32)
        zero_sb = const.tile([P, 1], F32)
        nc.vector.memset(eps_sb[:], LN_EPS)
        nc.vector.memset(zero_sb[:], 0.0)
        # touch Exp early so the ACT table load lands in the DMA bubble
        warm_sb = const.tile([1, 1], F32)
        nc.scalar.activation(warm_sb[:], zero_sb[0:1, :], AF.Exp, bias=zero_sb[0:1, :])
        nc.sync.dma_start(warm_d[:], warm_sb[:])
        nc.sync.dma_start(wq_sb[:], wq_d[:].rearrange("a p m -> p a m"))
        nc.sync.dma_start(wk_sb[:], wk_d[:].rearrange("a p m -> p a m"))
        nc.sync.dma_start(wv_sb[:], wv_d[:].rearrange("a p m -> p a m"))
        nc.sync.dma_start(wb_sb[:], wb_d[:])
        nc.sync.dma_start(wo_sb[:], wo_d[:])
        nc.sync.dma_start(id_sb[:], id_d[:])
        nc.sync.dma_start(cos_sb[:], cos_d[:])
        nc.sync.dma_start(sin_sb[:], sin_d[:])

        # long-lived activations
        xnT = big.tile([P, CB, N], BF16)  # xn transposed, c on partitions
        q_rope = big.tile([P, N], BF16)
        k_rope = big.tile([P, N], BF16)
        ktT = big.tile([P, N], BF16)  # heads contiguous on partitions
        v_sb = big.tile([P, NT, HPC * VW], BF16)  # [keys, tile, head|ones]
        outT_sc = big.tile([P, N], BF16)  # scaled attn out, d on partitions

        nc.gpsimd.memset(v_sb[:], 1.0)

        sps = ctx.enter_context(tc.tile_pool(name="sps", bufs=2, space="PSUM"))
        ep = ctx.enter_context(tc.tile_pool(name="ep", bufs=1))

        # ---- Front: LN + transpose + projections + rope + bilinear + v ----
        with ExitStack() as actx:
            xp = actx.enter_context(tc.tile_pool(name="xp", bufs=3))
            sp = actx.enter_context(tc.tile_pool(name="sp", bufs=8))
            xnp = actx.enter_context(tc.tile_pool(name="xnp", bufs=3))
            rtmp = actx.enter_context(tc.tile_pool(name="rtmp", bufs=2))
            tp = actx.enter_context(tc.tile_pool(name="tp", bufs=1, space="PSUM"))
            qkps = actx.enter_context(tc.tile_pool(name="qkps", bufs=1, space="PSUM"))
            vps = actx.enter_context(tc.tile_pool(name="vps", bufs=1, space="PSUM"))

            expT = {}

            def sim_exp_cell(j, ib):
                if j not in expT:
                    expT[j] = ep.tile(
                        [P, NIB, HPC, IB], BF16, tag=f"e{j}", name=f"e_{j}"
                    )
                e_j = expT[j]
                isl = slice(ib * IB, (ib + 1) * IB)
                ps_s = sps.tile([P, HPC, IB], F32, tag="sim", name="ps_s")
                for h in range(HPC):
                    hl = slice(h * DHEAD, (h + 1) * DHEAD)
                    nc.tensor.matmul(
                        ps_s[:, h, :],
                        ktT[hl, j * P : (j + 1) * P],
                        q_rope[hl, isl],
                        start=True,
                        stop=True,
                    )
                nc.scalar.activation(
                    e_j[:, ib, :, :], ps_s[:], AF.Exp, bias=zero_sb[:]
                )

            n_group = IB // P  # token tiles per i-block group (4)
            for tg in range(NT // n_group):
                sl = slice(tg * IB, (tg + 1) * IB)
                # LayerNorm + PE transpose for this group's token tiles
                for ti in range(n_group):
                    t = tg * n_group + ti
                    xt = xp.tile([P, DIM], F32, tag="x")
                    nc.sync.dma_start(xt[:], x_d[t * P : (t + 1) * P, :])
                    st = sp.tile([P, 2, 6], F32, tag="st")
                    nc.vector.bn_stats(st[:, 0, :], xt[:, 0:512])
                    nc.vector.bn_stats(st[:, 1, :], xt[:, 512:1024])
                    mv = sp.tile([P, 2], F32, tag="mv")
                    nc.vector.bn_aggr(mv[:], st[:])
                    rstd = sp.tile([P, 1], F32, tag="rstd")
                    nc.scalar.activation(rstd[:], mv[:, 1:2], AF.Sqrt, bias=eps_sb[:])
                    nc.vector.reciprocal(rstd[:], rstd[:])
                    xn = xnp.tile([P, DIM], BF16, tag="xn")
                    nc.vector.tensor_scalar(
                        xn[:], xt[:], mv[:, 0:1], rstd[:], ALU.subtract, ALU.mult
                    )
                    ps_t = [
                        tp.tile([P, 4, P], BF16, tag=f"t{half}", name=f"ps_t{half}")
                        for half in range(2)
                    ]
                    for cb in range(CB):
                        nc.tensor.transpose(
                            ps_t[cb // 4][:, cb % 4, :],
                            xn[:, cb * P : (cb + 1) * P],
                            id_sb[:],
                        )
                    nc.vector.tensor_copy(
                        xnT[:, 0:4, t * P : (t + 1) * P], ps_t[0][:]
                    )
                    nc.scalar.copy(
                        xnT[:, 4:8, t * P : (t + 1) * P], ps_t[1][:]
                    )
                # q/k projections + rope for this i-block
                for w_sb, dst in ((wq_sb, q_rope), (wk_sb, k_rope)):
                    ps_q = qkps.tile([P, IB], F32, tag="qk", name="ps_q")
                    for cb in range(CB):
                        nc.tensor.matmul(
                            ps_q[:],
                            w_sb[:, cb, :],
                            xnT[:, cb, sl],
                            start=(cb == 0),
                            stop=(cb == CB - 1),
                        )
                    tcos = rtmp.tile([P, IB], BF16, tag="tcos")
                    nc.vector.tensor_mul(tcos[:], ps_q[:], cos_sb[:, sl])
                    tsin = rtmp.tile([P, IB], BF16, tag="tsin")
                    for blk in range(4):
                        o0 = blk * 32
                        i0 = (blk ^ 1) * 32
                        nc.vector.tensor_mul(
                            tsin[o0 : o0 + 32, :],
                            ps_q[i0 : i0 + 32, :],
                            sin_sb[o0 : o0 + 32, sl],
                        )
                    nc.vector.tensor_add(dst[:, sl], tcos[:], tsin[:])
                # bilinear (block-diagonal, both heads) for this i-block
                ps_kt = qkps.tile([P, IB], F32, tag="qk", name="ps_kt")
                nc.tensor.matmul(
                    ps_kt[:], wb_sb[:], k_rope[:, sl], start=True, stop=True
                )
                nc.scalar.copy(ktT[:, sl], ps_kt[:])
                # v for this group's token tiles
                for ti in range(n_group):
                    t = tg * n_group + ti
                    ps_v = vps.tile([P, P], F32, tag="v")
                    for cb in range(CB):
                        nc.tensor.matmul(
                            ps_v[:],
                            xnT[:, cb, t * P : (t + 1) * P],
                            wv_sb[:, cb, :],
                            start=(cb == 0),
                            stop=(cb == CB - 1),
                        )
                    nc.scalar.copy(
                        v_sb[:, t, 0 : 2 * VW].rearrange("p (a b) -> p a b", a=2)[
                            :, :, 0:DHEAD
                        ],
                        ps_v[:].rearrange("p (a b) -> p a b", a=2),
                    )
                # QK^T + exp for every (j, ib) cell that just became ready
                for j in range(n_group * (tg + 1)):
                    if j >= n_group * tg:
                        ibs = range(tg + 1)
                    else:
                        ibs = [tg]
                    for ib in ibs:
                        if ib < NIB:
                            sim_exp_cell(j, ib)
            if debug_taps:
                nc.sync.dma_start(dbg["dbg_k"][:], k_rope[:])

        if debug_taps:
            nc.sync.dma_start(dbg["dbg_xnT"][:], xnT[:])
            nc.sync.dma_start(dbg["dbg_q"][:], q_rope[:])
            nc.sync.dma_start(dbg["dbg_kt"][:], ktT[:])
            nc.sync.dma_start(dbg["dbg_v"][:], v_sb[:])

        # ---- Attention: paired-head QK^T + exp stream, AV per head,
        # output projection interleaved per i-block group ----
        with ExitStack() as actx:
            avps = actx.enter_context(tc.tile_pool(name="avps", bufs=2, space="PSUM"))
            rp = actx.enter_context(tc.tile_pool(name="rp", bufs=1))
            op = actx.enter_context(tc.tile_pool(name="op", bufs=2))

            NG = 2 if NIB >= 2 else 1
            IPG = NIB // NG  # i-blocks per group
            GW = IPG * IB  # group width

            def wo_project(trange):
                """Output projection for token tiles in trange (needs outT_sc)."""
                for t in trange:
                    ps_o = sps.tile([P, HPC, IB], F32, tag="sim", name="ps_o")
                    for cc in range(DIM // IB):
                        nc.tensor.matmul(
                            ps_o[:, cc, :],
                            outT_sc[:, t * P : (t + 1) * P],
                            wo_sb[:, cc * IB : (cc + 1) * IB],
                            start=True,
                            stop=True,
                        )
                    o_lo = op.tile([P, IB], F32, tag="osb")
                    nc.vector.tensor_copy(o_lo[:], ps_o[:, 0, :])
                    nc.sync.dma_start(out_d[t * P : (t + 1) * P, 0:IB], o_lo[:])
                    o_hi = op.tile([P, IB], F32, tag="osb2")
                    nc.scalar.copy(o_hi[:], ps_o[:, 1, :])
                    nc.sync.dma_start(out_d[t * P : (t + 1) * P, IB:DIM], o_hi[:])

        with ExitStack() as actx:
            avps = actx.enter_context(tc.tile_pool(name="avps", bufs=2, space="PSUM"))
            rp = actx.enter_context(tc.tile_pool(name="rp", bufs=1))
            op = actx.enter_context(tc.tile_pool(name="op", bufs=2))
            for h in range(HPC):

                def av_mms(grp, ps_av):
                    for j in range(NT):
                        for il in range(IPG):
                            ib = grp * IPG + il
                            nc.tensor.matmul(
                                ps_av[:, il * IB : (il + 1) * IB],
                                v_sb[:, j, h * VW : (h + 1) * VW],
                                expT[j][:, ib, h, :],
                                start=(j == 0),
                                stop=(j == NT - 1),
                            )

                def av_scale(grp, ps_av):
                    for il in range(IPG):
                        gsl = slice(grp * GW + il * IB, grp * GW + (il + 1) * IB)
                        lsl = slice(il * IB, (il + 1) * IB)
                        rs_h = rp.tile([1, IB], F32, tag="rs")
                        nc.vector.tensor_copy(rs_h[:], ps_av[DHEAD : DHEAD + 1, lsl])
                        r_h = rp.tile([1, IB], F32, tag="r")
                        nc.vector.reciprocal_approx_fast(r_h[:], rs_h[:])
                        if debug_taps:
                            nc.sync.dma_start(dbg["dbg_r"][h, :, gsl], r_h[:])
                        rb_h = rp.tile([P, IB], F32, tag="rb")
                        nc.gpsimd.partition_broadcast(rb_h[:], r_h[:])
                        nc.vector.tensor_mul(
                            outT_sc[h * DHEAD : (h + 1) * DHEAD, gsl],
                            ps_av[0:DHEAD, lsl],
                            rb_h[h * DHEAD : (h + 1) * DHEAD, :],
                        )

                ps_avs = [
                    avps.tile([DHEAD + 1, GW], F32, tag="av", name=f"ps_av{g}")
                    for g in range(NG)
                ]
                for grp in range(NG):
                    av_mms(grp, ps_avs[grp])
                    av_scale(grp, ps_avs[grp])
                    if h == HPC - 1:
                        if debug_taps and grp == NG - 1:
                            nc.sync.dma_start(dbg["dbg_osc"][:], outT_sc[:])
                        tpg = NT // NG
                        wo_project(range(grp * tpg, (grp + 1) * tpg))

    nc.compile()
    return nc


def _rope_tables(N):
    theta = 1.0 / (ROPE_BASE ** (np.arange(0, DHEAD, 2, dtype=np.float64) / DHEAD))
    pos = np.arange(N, dtype=np.float64)
    freqs = pos[:, None] * theta[None, :]  # [N, 32]
    emb = np.concatenate([freqs, freqs], axis=-1)  # [N, 64]
    cos, sin = np.cos(emb), np.sin(emb)  # [N, 64]
    # per-head row order [evens(32) | odds(32)]:
    # out_even = q_even*cos[2r] - q_odd*sin[2r]
    # out_odd  = q_odd*cos[2r+1] + q_even*sin[2r+1]
    cosT = np.empty((DHEAD, N))
    sinT = np.empty((DHEAD, N))
    for r in range(32):
        cosT[r] = cos[:, 2 * r]
        cosT[32 + r] = cos[:, 2 * r + 1]
        sinT[r] = -sin[:, 2 * r]
        sinT[32 + r] = sin[:, 2 * r + 1]
    cosT2 = np.concatenate([cosT, cosT], axis=0)
    sinT2 = np.concatenate([sinT, sinT], axis=0)
    return (
        np.ascontiguousarray(cosT2.astype(ml_dtypes.bfloat16)),
        np.ascontiguousarray(sinT2.astype(ml_dtypes.bfloat16)),
    )


def _prep_inputs(x, gamma, Wq, Wkv, W_bilinear, Wo):
    """Slice/permute weights per core; returns list of 8 input dicts."""
    b, N, _ = x.shape
    x2d = np.ascontiguousarray(x.reshape(N, DIM)).astype(np.float32)
    cosT, sinT = _rope_tables(N)
    ident = np.eye(P, dtype=ml_dtypes.bfloat16)

    g = gamma.astype(np.float64)
    Wqg = g[:, None] * Wq.astype(np.float64) * (DHEAD**-0.5)
    Wkg = g[:, None] * Wkv[:, :INNER].astype(np.float64)
    Wvg = g[:, None] * Wkv[:, INNER:].astype(np.float64)

    perm = np.concatenate([_EVENS, _ODDS])  # within-head row order
    in_maps = []
    for c in range(NCORES):
        heads = [HPC * c + i for i in range(HPC)]
        gq = np.concatenate([h * DHEAD + perm for h in heads])
        vcols = np.concatenate(
            [np.arange(h * DHEAD, (h + 1) * DHEAD) for h in heads]
        )
        wq_c = Wqg[:, gq].astype(ml_dtypes.bfloat16).reshape(CB, P, P)
        wk_c = Wkg[:, gq].astype(ml_dtypes.bfloat16).reshape(CB, P, P)
        wv_c = Wvg[:, vcols].astype(ml_dtypes.bfloat16).reshape(CB, P, P)
        # block-diagonal bilinear: rows = k_rope rows, cols = ktT rows,
        # both in per-head [evens|odds] order
        wb_c = np.zeros((P, P), dtype=np.float64)
        for i, h in enumerate(heads):
            rows = np.arange(i * DHEAD, (i + 1) * DHEAD)
            wb_h = W_bilinear[h].astype(np.float64)[np.ix_(perm, perm)]
            wb_c[np.ix_(rows, rows)] = wb_h
        wo_c = Wo[vcols, :].astype(ml_dtypes.bfloat16)
        in_maps.append(
            {
                "x": x2d,
                "wq": np.ascontiguousarray(wq_c),
                "wk": np.ascontiguousarray(wk_c),
                "wv": np.ascontiguousarray(wv_c),
                "wb": np.ascontiguousarray(wb_c.astype(ml_dtypes.bfloat16)),
                "wo": np.ascontiguousarray(wo_c),
                "ident": ident,
                "cosT": cosT,
                "sinT": sinT,
            }
        )
    return in_maps


_NC_CACHE = {}


def _get_nc(N):
    if N not in _NC_CACHE:
        _NC_CACHE[N] = _build_nc(N)
    return _NC_CACHE[N]


def kernel(x, gamma, Wq, Wkv, W_bilinear, Wo, _trace=False, _trace_kwargs=None):
    b, N, dim = x.shape
    assert b == 1 and dim == DIM
    nc = _get_nc(N)
    in_maps = _prep_inputs(x, gamma, Wq, Wkv, W_bilinear, Wo)
    kw = {}
    if _trace:
        kw = {"trace": True, **(_trace_kwargs or {})}
    res = run_bass_kernel_spmd(nc, in_maps, core_ids=list(range(NCORES)), **kw)
    acc = np.zeros((N, DIM), dtype=np.float64)
    for c in range(NCORES):
        acc += res.results[c]["out"].astype(np.float64)
    out = acc.astype(np.float32).reshape(1, N, DIM)
    if _trace:
        return out, res
    return out


# revision 31
# speedup vs baseline: 1.0441x; 1.0196x over previous
"""Trainium2 Bass kernel for nn_Attention_28862180229709.

Head-sharded (2 heads/core x 8 cores) fused attention:
  LayerNorm -> Q/KV projections -> interleaved RoPE -> per-head bilinear K
  transform -> softmax(QK^T)V -> output projection (row-parallel Wo),
  host-side sum of the 8 partial outputs.

Layout strategy (per core):
  - xn is transposed on-chip (PE transpose) to xnT [c, n] so all projections
    contract c on the partition axis.
  - q/k are produced transposed ([d, n]) with each head's dims permuted to
    [evens | odds] so RoPE's interleaved pair-swap becomes 32-partition
    block swaps (partner = row ^ 32) done with strided DVE multiplies.
  - the per-head bilinear K transform is a single scattered block-diagonal
    weight matmul; QK^T runs as per-head K=64 matmul PAIRS packed into the
    PE array with tile_position row groups (a lone K=64 matmul never warms
    the PE clock gate - measured 427ns vs 216ns at N=512; a packed pair
    runs both heads in ~342ns).
  - scores are computed transposed (simT [keys, qrows]) so softmax
    normalization folds into the output side and attn @ V needs no
    transposes; row-sums come from an appended ones-column on V.
  - all matmuls run in bf16 (fp32 PSUM accumulation); LayerNorm, RoPE and
    softmax run in fp32 on DVE/ACT.
"""

import os
import sys

for _p in ("/opt/trn_rl_repo", "/root/.axon_site/_ro/trn_rl_repo"):
    if os.path.isdir(_p) and _p not in sys.path:
        sys.path.insert(0, _p)

from contextlib import ExitStack

import ml_dtypes
import numpy as np

import concourse.bacc as bacc
import concourse.tile as tile
from concourse import mybir
from concourse.bass_utils import run_bass_kernel_spmd

P = 128
DIM = 1024
HEADS = 16
DHEAD = 64
INNER = HEADS * DHEAD
NCORES = 8
HPC = HEADS // NCORES  # heads per core (2)
CB = DIM // P  # contraction chunks (8)
IB = 512  # i-block (psum bank) width
ROPE_BASE = 10000.0
LN_EPS = 1e-5

F32 = mybir.dt.float32
BF16 = mybir.dt.bfloat16
AF = mybir.ActivationFunctionType
ALU = mybir.AluOpType

# q/k row order: per head [evens(32) | odds(32)], heads contiguous.
_EVENS = np.arange(0, DHEAD, 2)
_ODDS = np.arange(1, DHEAD, 2)


def _build_nc(N, debug_taps=False):
    """Build the SPMD Bass program for sequence length N (tokens)."""
    NT = N // P  # token tiles
    NIB = N // IB  # i-blocks
    assert N % IB == 0

    nc = bacc.Bacc("TRN2", target_bir_lowering=False, debug=False, dynamic_dma_scratch_size=2048)

    x_d = nc.dram_tensor("x", (N, DIM), F32, kind="ExternalInput")
    wq_d = nc.dram_tensor("wq", (CB, P, P), BF16, kind="ExternalInput")
    wk_d = nc.dram_tensor("wk", (CB, P, P), BF16, kind="ExternalInput")
    wv_d = nc.dram_tensor("wv", (CB, P, P), BF16, kind="ExternalInput")
    wb_d = nc.dram_tensor("wb", (P, P), BF16, kind="ExternalInput")
    wo_d = nc.dram_tensor("wo", (P, DIM), BF16, kind="ExternalInput")
    id_d = nc.dram_tensor("ident", (P, P), BF16, kind="ExternalInput")
    cos_d = nc.dram_tensor("cosT", (P, N), BF16, kind="ExternalInput")
    sin_d = nc.dram_tensor("sinT", (P, N), BF16, kind="ExternalInput")
    out_d = nc.dram_tensor("out", (N, DIM), F32, kind="ExternalOutput")
    warm_d = nc.dram_tensor("warm", (1, 1), F32, kind="ExternalOutput")
    if debug_taps:
        dbg = {
            "dbg_xnT": nc.dram_tensor("dbg_xnT", (P, CB, N), BF16, kind="ExternalOutput"),
            "dbg_q": nc.dram_tensor("dbg_q", (P, N), BF16, kind="ExternalOutput"),
            "dbg_k": nc.dram_tensor("dbg_k", (P, N), BF16, kind="ExternalOutput"),
            "dbg_kt": nc.dram_tensor("dbg_kt", (P, N), BF16, kind="ExternalOutput"),
            "dbg_r": nc.dram_tensor("dbg_r", (HPC, 1, N), F32, kind="ExternalOutput"),
            "dbg_osc": nc.dram_tensor("dbg_osc", (P, N), BF16, kind="ExternalOutput"),
            "dbg_v": nc.dram_tensor("dbg_v", (P, NT, 2 * (DHEAD + 1)), BF16, kind="ExternalOutput"),
        }

    VW = DHEAD + 1

    with tile.TileContext(nc) as tc, ExitStack() as ctx:
        const = ctx.enter_context(tc.tile_pool(name="const", bufs=1))
        big = ctx.enter_context(tc.tile_pool(name="big", bufs=1))

        wq_sb = const.tile([P, CB, P], BF16)
        wk_sb = const.tile([P, CB, P], BF16)
        wv_sb = const.tile([P, CB, P], BF16)
        wb_sb = const.tile([P, P], BF16)
        wo_sb = const.tile([P, DIM], BF16)
        id_sb = const.tile([P, P], BF16)
        cos_sb = const.tile([P, N], BF16)
        sin_sb = const.tile([P, N], BF16)
        eps_sb = const.tile([P, 1], F32)
        zero_sb = const.tile([P, 1], F32)
        nc.vector.memset(eps_sb[:], LN_EPS)
        nc.vector.memset(zero_sb[:], 0.0)
        # touch Exp early so the ACT table load lands in the DMA bubble
        warm_sb = const.tile([1, 1], F32)
        nc.scalar.activation(warm_sb[:], zero_sb[0:1, :], AF.Exp, bias=zero_sb[0:1, :])
        nc.sync.dma_start(warm_d[:], warm_sb[:])
        nc.sync.dma_start(wq_sb[:], wq_d[:].rearrange("a p m -> p a m"))
        nc.sync.dma_start(wk_sb[:], wk_d[:].rearrange("a p m -> p a m"))
        nc.sync.dma_start(wv_sb[:], wv_d[:].rearrange("a p m -> p a m"))
        nc.sync.dma_start(wb_sb[:], wb_d[:])
        nc.sync.dma_start(wo_sb[:], wo_d[:])
        nc.sync.dma_start(id_sb[:], id_d[:])
        nc.sync.dma_start(cos_sb[:], cos_d[:])
        nc.sync.dma_start(sin_sb[:], sin_d[:])

        # long-lived activations
        xnT = big.tile([P, CB, N], BF16)  # xn transposed, c on partitions
        q_rope = big.tile([P, N], BF16)
        k_rope = big.tile([P, N], BF16)
        ktT = big.tile([P, N], BF16)  # heads contiguous on partitions
        v_sb = big.tile([P, NT, HPC * VW], BF16)  # [keys, tile, head|ones]
        outT_sc = big.tile([P, N], BF16)  # scaled attn out, d on partitions

        nc.gpsimd.memset(v_sb[:], 1.0)

        sps = ctx.enter_context(tc.tile_pool(name="sps", bufs=2, space="PSUM"))
        ep = ctx.enter_context(tc.tile_pool(name="ep", bufs=1))

        # ---- Front: LN + transpose + projections + rope + bilinear + v ----
        with ExitStack() as actx:
            xp = actx.enter_context(tc.tile_pool(name="xp", bufs=3))
            sp = actx.enter_context(tc.tile_pool(name="sp", bufs=8))
            xnp = actx.enter_context(tc.tile_pool(name="xnp", bufs=3))
            rtmp = actx.enter_context(tc.tile_pool(name="rtmp", bufs=2))
            tp = actx.enter_context(tc.tile_pool(name="tp", bufs=1, space="PSUM"))
            qkps = actx.enter_context(tc.tile_pool(name="qkps", bufs=1, space="PSUM"))
            vps = actx.enter_context(tc.tile_pool(name="vps", bufs=1, space="PSUM"))

            expT = {}

            def sim_exp_cell(j, ib):
                if j not in expT:
                    expT[j] = ep.tile(
                        [P, NIB, HPC, IB], BF16, tag=f"e{j}", name=f"e_{j}"
                    )
                e_j = expT[j]
                isl = slice(ib * IB, (ib + 1) * IB)
                ps_s = sps.tile([P, HPC, IB], F32, tag="sim", name="ps_s")
                for h in range(HPC):
                    hl = slice(h * DHEAD, (h + 1) * DHEAD)
                    nc.tensor.matmul(
                        ps_s[:, h, :],
                        ktT[hl, j * P : (j + 1) * P],
                        q_rope[hl, isl],
                        start=True,
                        stop=True,
                    )
                nc.scalar.activation(
                    e_j[:, ib, :, :], ps_s[:], AF.Exp, bias=zero_sb[:]
                )

            n_group = IB // P  # token tiles per i-block group (4)
            for tg in range(NT // n_group):
                sl = slice(tg * IB, (tg + 1) * IB)
                # LayerNorm + PE transpose for this group's token tiles
                for ti in range(n_group):
                    t = tg * n_group + ti
                    xt = xp.tile([P, DIM], F32, tag="x")
                    nc.sync.dma_start(xt[:], x_d[t * P : (t + 1) * P, :])
                    st = sp.tile([P, 2, 6], F32, tag="st")
                    nc.vector.bn_stats(st[:, 0, :], xt[:, 0:512])
                    nc.vector.bn_stats(st[:, 1, :], xt[:, 512:1024])
                    mv = sp.tile([P, 2], F32, tag="mv")
                    nc.vector.bn_aggr(mv[:], st[:])
                    rstd = sp.tile([P, 1], F32, tag="rstd")
                    nc.scalar.activation(rstd[:], mv[:, 1:2], AF.Sqrt, bias=eps_sb[:])
                    nc.vector.reciprocal(rstd[:], rstd[:])
                    xn = xnp.tile([P, DIM], BF16, tag="xn")
                    nc.vector.tensor_scalar(
                        xn[:], xt[:], mv[:, 0:1], rstd[:], ALU.subtract, ALU.mult
                    )
                    ps_t = [
                        tp.tile([P, 4, P], BF16, tag=f"t{half}", name=f"ps_t{half}")
                        for half in range(2)
                    ]
                    for cb in range(CB):
                        nc.tensor.transpose(
                            ps_t[cb // 4][:, cb % 4, :],
                            xn[:, cb * P : (cb + 1) * P],
                            id_sb[:],
                        )
                    nc.vector.tensor_copy(
                        xnT[:, 0:4, t * P : (t + 1) * P], ps_t[0][:]
                    )
                    nc.scalar.copy(
                        xnT[:, 4:8, t * P : (t + 1) * P], ps_t[1][:]
                    )
                # q/k projections + rope for this i-block
                for w_sb, dst in ((wq_sb, q_rope), (wk_sb, k_rope)):
                    ps_q = qkps.tile([P, IB], F32, tag="qk", name="ps_q")
                    for cb in range(CB):
                        nc.tensor.matmul(
                            ps_q[:],
                            w_sb[:, cb, :],
                            xnT[:, cb, sl],
                            start=(cb == 0),
                            stop=(cb == CB - 1),
                        )
                    tcos = rtmp.tile([P, IB], BF16, tag="tcos")
                    nc.vector.tensor_mul(tcos[:], ps_q[:], cos_sb[:, sl])
                    tsin = rtmp.tile([P, IB], BF16, tag="tsin")
                    for blk in range(4):
                        o0 = blk * 32
                        i0 = (blk ^ 1) * 32
                        nc.vector.tensor_mul(
                            tsin[o0 : o0 + 32, :],
                            ps_q[i0 : i0 + 32, :],
                            sin_sb[o0 : o0 + 32, sl],
                        )
                    nc.vector.tensor_add(dst[:, sl], tcos[:], tsin[:])
                # bilinear (block-diagonal, both heads) for this i-block
                ps_kt = qkps.tile([P, IB], F32, tag="qk", name="ps_kt")
                nc.tensor.matmul(
                    ps_kt[:], wb_sb[:], k_rope[:, sl], start=True, stop=True
                )
                nc.scalar.copy(ktT[:, sl], ps_kt[:])
                # v for this group's token tiles
                for ti in range(n_group):
                    t = tg * n_group + ti
                    ps_v = vps.tile([P, P], F32, tag="v")
                    for cb in range(CB):
                        nc.tensor.matmul(
                            ps_v[:],
                            xnT[:, cb, t * P : (t + 1) * P],
                            wv_sb[:, cb, :],
                            start=(cb == 0),
                            stop=(cb == CB - 1),
                        )
                    nc.scalar.copy(
                        v_sb[:, t, 0 : 2 * VW].rearrange("p (a b) -> p a b", a=2)[
                            :, :, 0:DHEAD
                        ],
                        ps_v[:].rearrange("p (a b) -> p a b", a=2),
                    )
                # QK^T + exp for every (j, ib) cell that just became ready
                for j in range(n_group * (tg + 1)):
                    if j >= n_group * tg:
                        ibs = range(tg + 1)
                    else:
                        ibs = [tg]
                    for ib in ibs:
                        if ib < NIB:
                            sim_exp_cell(j, ib)
            if debug_taps:
                nc.sync.dma_start(dbg["dbg_k"][:], k_rope[:])

        if debug_taps:
            nc.sync.dma_start(dbg["dbg_xnT"][:], xnT[:])
            nc.sync.dma_start(dbg["dbg_q"][:], q_rope[:])
            nc.sync.dma_start(dbg["dbg_kt"][:], ktT[:])
            nc.sync.dma_start(dbg["dbg_v"][:], v_sb[:])

        # ---- Attention: paired-head QK^T + exp stream, AV per head,
        # output projection interleaved per i-block group ----
        with ExitStack() as actx:
            avps = actx.enter_context(tc.tile_pool(name="avps", bufs=2, space="PSUM"))
            rp = actx.enter_context(tc.tile_pool(name="rp", bufs=2))
            op = actx.enter_context(tc.tile_pool(name="op", bufs=3))

            NG = 2 if NIB >= 2 else 1
            IPG = NIB // NG  # i-blocks per group
            GW = IPG * IB  # group width

            def wo_project(trange):
                """Output projection for token tiles in trange (needs outT_sc)."""
                for t in trange:
                    ps_o = sps.tile([P, HPC, IB], F32, tag="sim", name="ps_o")
                    for cc in range(DIM // IB):
                        nc.tensor.matmul(
                            ps_o[:, cc, :],
                            outT_sc[:, t * P : (t + 1) * P],
                            wo_sb[:, cc * IB : (cc + 1) * IB],
                            start=True,
                            stop=True,
                        )
                    o_lo = op.tile([P, IB], F32, tag="osb")
                    nc.vector.tensor_copy(o_lo[:], ps_o[:, 0, :])
                    nc.sync.dma_start(out_d[t * P : (t + 1) * P, 0:IB], o_lo[:])
                    o_hi = op.tile([P, IB], F32, tag="osb2")
                    nc.scalar.copy(o_hi[:], ps_o[:, 1, :])
                    nc.sync.dma_start(out_d[t * P : (t + 1) * P, IB:DIM], o_hi[:])

        with ExitStack() as actx:
            avps = actx.enter_context(tc.tile_pool(name="avps", bufs=2, space="PSUM"))
            rp = actx.enter_context(tc.tile_pool(name="rp", bufs=2))
            op = actx.enter_context(tc.tile_pool(name="op", bufs=3))
            for h in range(HPC):

                def av_mms(grp, ps_av):
                    for j in range(NT):
                        for il in range(IPG):
                            ib = grp * IPG + il
                            nc.tensor.matmul(
                                ps_av[:, il * IB : (il + 1) * IB],
                                v_sb[:, j, h * VW : (h + 1) * VW],
                                expT[j][:, ib, h, :],
                                start=(j == 0),
                                stop=(j == NT - 1),
                            )

                def av_scale(grp, ps_av):
                    for il in range(IPG):
                        gsl = slice(grp * GW + il * IB, grp * GW + (il + 1) * IB)
                        lsl = slice(il * IB, (il + 1) * IB)
                        rs_h = rp.tile([1, IB], F32, tag="rs")
                        nc.vector.tensor_copy(rs_h[:], ps_av[DHEAD : DHEAD + 1, lsl])
                        r_h = rp.tile([1, IB], F32, tag="r")
                        nc.vector.reciprocal_approx_fast(r_h[:], rs_h[:])
                        if debug_taps:
                            nc.sync.dma_start(dbg["dbg_r"][h, :, gsl], r_h[:])
                        rb_h = rp.tile([P, IB], F32, tag="rb")
                        nc.gpsimd.partition_broadcast(rb_h[:], r_h[:])
                        nc.vector.tensor_mul(
                            outT_sc[h * DHEAD : (h + 1) * DHEAD, gsl],
                            ps_av[0:DHEAD, lsl],
                            rb_h[h * DHEAD : (h + 1) * DHEAD, :],
                        )

                ps_avs = [
                    avps.tile([DHEAD + 1, GW], F32, tag="av", name=f"ps_av{g}")
                    for g in range(NG)
                ]
                for grp in range(NG):
                    av_mms(grp, ps_avs[grp])
                    av_scale(grp, ps_avs[grp])
                    if h == HPC - 1:
                        if debug_taps and grp == NG - 1:
                            nc.sync.dma_start(dbg["dbg_osc"][:], outT_sc[:])
                        tpg = NT // NG
                        wo_project(range(grp * tpg, (grp + 1) * tpg))

    nc.compile()
    return nc


def _rope_tables(N):
    theta = 1.0 / (ROPE_BASE ** (np.arange(0, DHEAD, 2, dtype=np.float64) / DHEAD))
    pos = np.arange(N, dtype=np.float64)
    freqs = pos[:, None] * theta[None, :]  # [N, 32]
    emb = np.concatenate([freqs, freqs], axis=-1)  # [N, 64]
    cos, sin = np.cos(emb), np.sin(emb)  # [N, 64]
    # per-head row order [evens(32) | odds(32)]:
    # out_even = q_even*cos[2r] - q_odd*sin[2r]
    # out_odd  = q_odd*cos[2r+1] + q_even*sin[2r+1]
    cosT = np.empty((DHEAD, N))
    sinT = np.empty((DHEAD, N))
    for r in range(32):
        cosT[r] = cos[:, 2 * r]
        cosT[32 + r] = cos[:, 2 * r + 1]
        sinT[r] = -sin[:, 2 * r]
        sinT[32 + r] = sin[:, 2 * r + 1]
    cosT2 = np.concatenate([cosT, cosT], axis=0)
    sinT2 = np.concatenate([sinT, sinT], axis=0)
    return (
        np.ascontiguousarray(cosT2.astype(ml_dtypes.bfloat16)),
        np.ascontiguousarray(sinT2.astype(ml_dtypes.bfloat16)),
    )


def _prep_inputs(x, gamma, Wq, Wkv, W_bilinear, Wo):
    """Slice/permute weights per core; returns list of 8 input dicts."""
    b, N, _ = x.shape
    x2d = np.ascontiguousarray(x.reshape(N, DIM)).astype(np.float32)
    cosT, sinT = _rope_tables(N)
    ident = np.eye(P, dtype=ml_dtypes.bfloat16)

    g = gamma.astype(np.float64)
    Wqg = g[:, None] * Wq.astype(np.float64) * (DHEAD**-0.5)
    Wkg = g[:, None] * Wkv[:, :INNER].astype(np.float64)
    Wvg = g[:, None] * Wkv[:, INNER:].astype(np.float64)

    perm = np.concatenate([_EVENS, _ODDS])  # within-head row order
    in_maps = []
    for c in range(NCORES):
        heads = [HPC * c + i for i in range(HPC)]
        gq = np.concatenate([h * DHEAD + perm for h in heads])
        vcols = np.concatenate(
            [np.arange(h * DHEAD, (h + 1) * DHEAD) for h in heads]
        )
        wq_c = Wqg[:, gq].astype(ml_dtypes.bfloat16).reshape(CB, P, P)
        wk_c = Wkg[:, gq].astype(ml_dtypes.bfloat16).reshape(CB, P, P)
        wv_c = Wvg[:, vcols].astype(ml_dtypes.bfloat16).reshape(CB, P, P)
        # block-diagonal bilinear: rows = k_rope rows, cols = ktT rows,
        # both in per-head [evens|odds] order
        wb_c = np.zeros((P, P), dtype=np.float64)
        for i, h in enumerate(heads):
            rows = np.arange(i * DHEAD, (i + 1) * DHEAD)
            wb_h = W_bilinear[h].astype(np.float64)[np.ix_(perm, perm)]
            wb_c[np.ix_(rows, rows)] = wb_h
        wo_c = Wo[vcols, :].astype(ml_dtypes.bfloat16)
        in_maps.append(
            {
                "x": x2d,
                "wq": np.ascontiguousarray(wq_c),
                "wk": np.ascontiguousarray(wk_c),
                "wv": np.ascontiguousarray(wv_c),
                "wb": np.ascontiguousarray(wb_c.astype(ml_dtypes.bfloat16)),
                "wo": np.ascontiguousarray(wo_c),
                "ident": ident,
                "cosT": cosT,
                "sinT": sinT,
            }
        )
    return in_maps


_NC_CACHE = {}


def _get_nc(N):
    if N not in _NC_CACHE:
        _NC_CACHE[N] = _build_nc(N)
    return _NC_CACHE[N]


def kernel(x, gamma, Wq, Wkv, W_bilinear, Wo, _trace=False, _trace_kwargs=None):
    b, N, dim = x.shape
    assert b == 1 and dim == DIM
    nc = _get_nc(N)
    in_maps = _prep_inputs(x, gamma, Wq, Wkv, W_bilinear, Wo)
    kw = {}
    if _trace:
        kw = {"trace": True, **(_trace_kwargs or {})}
    res = run_bass_kernel_spmd(nc, in_maps, core_ids=list(range(NCORES)), **kw)
    acc = np.zeros((N, DIM), dtype=np.float64)
    for c in range(NCORES):
        acc += res.results[c]["out"].astype(np.float64)
    out = acc.astype(np.float32).reshape(1, N, DIM)
    if _trace:
        return out, res
    return out
